# revision 5
# baseline (speedup 1.0000x reference)
"""Binary-weight dense layer on 8 trn2 NeuronCores.

Computes out[b,s,f] = scale * sum_i x[b,s,i] * (kernel[i,f] ? +1 : -1)
for x [4, 4096, 1024] f32, kernel [1024, 1024] bool, scale scalar f32.

Strategy: data-parallel over the 16384 rows (2048 rows/core).  Host-side
prep transposes each x shard to [K, rows] bf16 so SBUF tiles land in
matmul layout with fully-contiguous DMA lines, and folds scale into the
+-1 weights (exact in bf16 for power-of-two scales).  On-chip: pure
bf16 matmul accumulating fp32 in PSUM, DVE copy to SBUF, DMA out.
"""

import numpy as np
import ml_dtypes

import concourse.bacc as bacc
import concourse.mybir as mybir
import concourse.tile as tile
from concourse.bass_utils import run_bass_kernel_spmd

N_CORES = 8
B, S, K, N = 4, 4096, 1024, 1024
ROWS = B * S                    # 16384
ROWS_PER_CORE = ROWS // N_CORES  # 2048
P = 128                         # partitions
KT = K // P                     # 8 contraction subtiles
MT = ROWS_PER_CORE // P         # 16 row tiles per core
NHALF = 512                     # one PSUM bank of f32

_module_cache = {}


def build_module():
    nc = bacc.Bacc(None)
    xt = nc.dram_tensor("xt", [K, ROWS_PER_CORE], mybir.dt.bfloat16,
                        kind="ExternalInput")
    w = nc.dram_tensor("w", [K, N], mybir.dt.bfloat16, kind="ExternalInput")
    out = nc.dram_tensor("out", [ROWS_PER_CORE, N], mybir.dt.float32,
                         kind="ExternalOutput")

    with tile.TileContext(nc) as tc:
        with (
            tc.tile_pool(name="persist", bufs=1) as persist,
            tc.tile_pool(name="psum", bufs=4, space="PSUM") as ps_pool,
            tc.tile_pool(name="outp", bufs=3) as out_pool,
        ):
            w_tiles = []
            x_tiles = []
            for k in range(KT):
                wt = persist.tile([P, N], mybir.dt.bfloat16, tag=f"w{k}")
                nc.sync.dma_start(out=wt, in_=w[k * P:(k + 1) * P, :])
                w_tiles.append(wt)
            for k in range(KT):
                xtile = persist.tile([P, ROWS_PER_CORE], mybir.dt.bfloat16,
                                     tag=f"x{k}")
                nc.sync.dma_start(out=xtile, in_=xt[k * P:(k + 1) * P, :])
                x_tiles.append(xtile)

            for m in range(MT):
                ps = ps_pool.tile([P, N], mybir.dt.float32, tag="ps", bufs=2)
                for k in range(KT):
                    lhsT = x_tiles[k][:, m * P:(m + 1) * P]
                    nc.tensor.matmul(ps[:, 0:NHALF], lhsT,
                                     w_tiles[k][:, 0:NHALF],
                                     start=(k == 0), stop=(k == KT - 1))
                    nc.tensor.matmul(ps[:, NHALF:N], lhsT,
                                     w_tiles[k][:, NHALF:N],
                                     start=(k == 0), stop=(k == KT - 1))
                ot = out_pool.tile([P, N], mybir.dt.float32, tag="ot")
                nc.vector.tensor_copy(ot, ps)
                nc.sync.dma_start(out=out[m * P:(m + 1) * P, :], in_=ot)
    nc.finalize()
    return nc


def get_module():
    if "nc" not in _module_cache:
        _module_cache["nc"] = build_module()
    return _module_cache["nc"]


def _prepare_in_maps(x, kernel, scale):
    bf16 = ml_dtypes.bfloat16
    x2d = np.asarray(x, dtype=np.float32).reshape(ROWS, K)
    scale = np.float32(scale)
    w_signed = np.where(np.asarray(kernel, dtype=bool), scale, -scale)
    w_bf16 = np.ascontiguousarray(w_signed.astype(bf16))
    in_maps = []
    for c in range(N_CORES):
        shard = x2d[c * ROWS_PER_CORE:(c + 1) * ROWS_PER_CORE]
        xt_c = np.ascontiguousarray(shard.T.astype(bf16))
        in_maps.append({"xt": xt_c, "w": w_bf16})
    return in_maps


def kernel(x, kernel, scale):
    nc = get_module()
    in_maps = _prepare_in_maps(x, kernel, scale)
    res = run_bass_kernel_spmd(nc, in_maps, core_ids=list(range(N_CORES)))
    out = np.concatenate([r["out"] for r in res.results], axis=0)
    return out.reshape(B, S, N)


# revision 7
# speedup vs baseline: 1.1367x; 1.1367x over previous
"""Binary-weight dense layer on 8 trn2 NeuronCores.

Computes out[b,s,f] = scale * sum_i x[b,s,i] * (kernel[i,f] ? +1 : -1)
for x [4, 4096, 1024] f32, kernel [1024, 1024] bool, scale scalar f32.

Strategy: data-parallel over the 16384 rows (2048 rows/core).  Host-side
prep transposes each x shard to [K, rows] bf16 so SBUF tiles land in
matmul layout with fully-contiguous DMA lines, and folds scale into the
+-1 weights (exact in bf16 for power-of-two scales).  On-chip: pure
bf16 matmul accumulating fp32 in PSUM, DVE copy to SBUF, DMA out.
"""

import numpy as np
import ml_dtypes

import concourse.bacc as bacc
import concourse.mybir as mybir
import concourse.tile as tile
from concourse.bass_utils import run_bass_kernel_spmd

N_CORES = 8
B, S, K, N = 4, 4096, 1024, 1024
ROWS = B * S                    # 16384
ROWS_PER_CORE = ROWS // N_CORES  # 2048
P = 128                         # partitions
KT = K // P                     # 8 contraction subtiles
MT = ROWS_PER_CORE // P         # 16 row tiles per core
NHALF = 512                     # one PSUM bank of f32

_module_cache = {}


def build_module():
    nc = bacc.Bacc(None)
    xt = nc.dram_tensor("xt", [K, ROWS_PER_CORE], mybir.dt.bfloat16,
                        kind="ExternalInput")
    w = nc.dram_tensor("w", [K, N], mybir.dt.bfloat16, kind="ExternalInput")
    out = nc.dram_tensor("out", [ROWS_PER_CORE, N], mybir.dt.float32,
                         kind="ExternalOutput")

    HROWS = ROWS_PER_CORE // 2   # 1024 rows per x half-chunk
    HM = HROWS // P              # 8 m-tiles per half
    G0 = 4                       # m-tiles processed k-major during load phase

    with tile.TileContext(nc) as tc:
        with (
            tc.tile_pool(name="persist", bufs=1) as persist,
            tc.tile_pool(name="psum", bufs=1, space="PSUM") as ps_pool,
            tc.tile_pool(name="outp", bufs=3) as out_pool,
        ):
            # Interleave w_k with the first-half x chunks so PE can start
            # the k-accumulation of the first m-group as chunks land.
            w_tiles = [None] * KT
            x_chunks = [[None] * KT for _ in range(2)]
            for k in range(KT):
                wt = persist.tile([P, N], mybir.dt.bfloat16, tag=f"w{k}")
                nc.sync.dma_start(out=wt, in_=w[k * P:(k + 1) * P, :])
                w_tiles[k] = wt
                xc = persist.tile([P, HROWS], mybir.dt.bfloat16, tag=f"x{k}h0")
                nc.sync.dma_start(out=xc,
                                  in_=xt[k * P:(k + 1) * P, 0:HROWS])
                x_chunks[0][k] = xc
            for k in range(KT):
                xc = persist.tile([P, HROWS], mybir.dt.bfloat16, tag=f"x{k}h1")
                nc.sync.dma_start(out=xc,
                                  in_=xt[k * P:(k + 1) * P, HROWS:ROWS_PER_CORE])
                x_chunks[1][k] = xc

            ps_tiles = {}

            def mm(m, k):
                h, off = divmod(m, HM)
                lhsT = x_chunks[h][k][:, off * P:(off + 1) * P]
                ps = ps_tiles[m % G0]
                nc.tensor.matmul(ps[:, 0:NHALF], lhsT,
                                 w_tiles[k][:, 0:NHALF],
                                 start=(k == 0), stop=(k == KT - 1))
                nc.tensor.matmul(ps[:, NHALF:N], lhsT,
                                 w_tiles[k][:, NHALF:N],
                                 start=(k == 0), stop=(k == KT - 1))

            def evict(m):
                ot = out_pool.tile([P, N], mybir.dt.float32, tag="ot")
                nc.vector.tensor_copy(ot, ps_tiles[m % G0])
                # second HWDGE ring (ACT) so stores don't queue behind loads
                nc.scalar.dma_start(out=out[m * P:(m + 1) * P, :], in_=ot)

            # Phase 1: first G0 m-tiles k-major, consuming chunks as they
            # arrive from DMA.
            for m in range(G0):
                ps_tiles[m] = ps_pool.tile([P, N], mybir.dt.float32,
                                           tag=f"ps{m}", name=f"ps{m}")
            for k in range(KT):
                for m in range(G0):
                    mm(m, k)
            for m in range(G0):
                evict(m)

            # Phase 2: remaining m-tiles m-major (inputs now resident),
            # copy-out pipelined with the next tile's matmuls.
            for m in range(G0, MT):
                ps_tiles[m % G0] = ps_pool.tile([P, N], mybir.dt.float32,
                                                tag=f"ps{m % G0}",
                                                name=f"ps{m}")
                for k in range(KT):
                    mm(m, k)
                evict(m)
    nc.finalize()
    return nc


def get_module():
    if "nc" not in _module_cache:
        _module_cache["nc"] = build_module()
    return _module_cache["nc"]


def _prepare_in_maps(x, kernel, scale):
    bf16 = ml_dtypes.bfloat16
    x2d = np.asarray(x, dtype=np.float32).reshape(ROWS, K)
    scale = np.float32(scale)
    w_signed = np.where(np.asarray(kernel, dtype=bool), scale, -scale)
    w_bf16 = np.ascontiguousarray(w_signed.astype(bf16))
    in_maps = []
    for c in range(N_CORES):
        shard = x2d[c * ROWS_PER_CORE:(c + 1) * ROWS_PER_CORE]
        xt_c = np.ascontiguousarray(shard.T.astype(bf16))
        in_maps.append({"xt": xt_c, "w": w_bf16})
    return in_maps


def kernel(x, kernel, scale):
    nc = get_module()
    in_maps = _prepare_in_maps(x, kernel, scale)
    res = run_bass_kernel_spmd(nc, in_maps, core_ids=list(range(N_CORES)))
    out = np.concatenate([r["out"] for r in res.results], axis=0)
    return out.reshape(B, S, N)


# revision 8
# speedup vs baseline: 1.1598x; 1.0203x over previous
"""Binary-weight dense layer on 8 trn2 NeuronCores.

Computes out[b,s,f] = scale * sum_i x[b,s,i] * (kernel[i,f] ? +1 : -1)
for x [4, 4096, 1024] f32, kernel [1024, 1024] bool, scale scalar f32.

Strategy: data-parallel over the 16384 rows (2048 rows/core).  Host-side
prep transposes each x shard to [K, rows] bf16 so SBUF tiles land in
matmul layout with fully-contiguous DMA lines, and folds scale into the
+-1 weights (exact in bf16 for power-of-two scales).  On-chip: pure
bf16 matmul accumulating fp32 in PSUM, DVE copy to SBUF, DMA out.
"""

import numpy as np
import ml_dtypes

import concourse.bacc as bacc
import concourse.mybir as mybir
import concourse.tile as tile
from concourse.bass_utils import run_bass_kernel_spmd

N_CORES = 8
B, S, K, N = 4, 4096, 1024, 1024
ROWS = B * S                    # 16384
ROWS_PER_CORE = ROWS // N_CORES  # 2048
P = 128                         # partitions
KT = K // P                     # 8 contraction subtiles
MT = ROWS_PER_CORE // P         # 16 row tiles per core
NHALF = 512                     # one PSUM bank of f32

_module_cache = {}


def build_module():
    nc = bacc.Bacc(None)
    xt = nc.dram_tensor("xt", [K, ROWS_PER_CORE], mybir.dt.bfloat16,
                        kind="ExternalInput")
    w = nc.dram_tensor("w", [K, N], mybir.dt.bfloat16, kind="ExternalInput")
    out = nc.dram_tensor("out", [ROWS_PER_CORE, N], mybir.dt.float32,
                         kind="ExternalOutput")

    HROWS = ROWS_PER_CORE // 2   # 1024 rows per x half-chunk
    HM = HROWS // P              # 8 m-tiles per half
    G0 = 4                       # m-tiles processed k-major during load phase

    with tile.TileContext(nc) as tc:
        with (
            tc.tile_pool(name="persist", bufs=1) as persist,
            tc.tile_pool(name="psum", bufs=1, space="PSUM") as ps_pool,
            tc.tile_pool(name="outp", bufs=3) as out_pool,
        ):
            # Load w in N-halves and x in row-halves, x piece first, so the
            # first matmul's dependencies are small and early in the SDMA
            # round-robin (big batches all complete together at the end).
            w_half = [[None] * 2 for _ in range(KT)]
            x_chunks = [[None] * KT for _ in range(2)]
            for k in range(KT):
                xc = persist.tile([P, HROWS], mybir.dt.bfloat16, tag=f"x{k}h0")
                nc.sync.dma_start(out=xc,
                                  in_=xt[k * P:(k + 1) * P, 0:HROWS])
                x_chunks[0][k] = xc
                for j in range(2):
                    wt = persist.tile([P, NHALF], mybir.dt.bfloat16,
                                      tag=f"w{k}j{j}", name=f"w{k}j{j}")
                    nc.sync.dma_start(
                        out=wt, in_=w[k * P:(k + 1) * P,
                                      j * NHALF:(j + 1) * NHALF])
                    w_half[k][j] = wt
            for k in range(KT):
                xc = persist.tile([P, HROWS], mybir.dt.bfloat16, tag=f"x{k}h1")
                nc.sync.dma_start(out=xc,
                                  in_=xt[k * P:(k + 1) * P, HROWS:ROWS_PER_CORE])
                x_chunks[1][k] = xc

            ps_tiles = {}

            def mm(m, k):
                h, off = divmod(m, HM)
                lhsT = x_chunks[h][k][:, off * P:(off + 1) * P]
                ps = ps_tiles[m % G0]
                nc.tensor.matmul(ps[:, 0:NHALF], lhsT, w_half[k][0],
                                 start=(k == 0), stop=(k == KT - 1))
                nc.tensor.matmul(ps[:, NHALF:N], lhsT, w_half[k][1],
                                 start=(k == 0), stop=(k == KT - 1))

            def evict(m):
                ot = out_pool.tile([P, N], mybir.dt.float32, tag="ot")
                nc.vector.tensor_copy(ot, ps_tiles[m % G0])
                # second HWDGE ring (ACT) so stores don't queue behind loads
                nc.scalar.dma_start(out=out[m * P:(m + 1) * P, :], in_=ot)

            # Phase 1: first G0 m-tiles k-major, consuming chunks as they
            # arrive from DMA.
            for m in range(G0):
                ps_tiles[m] = ps_pool.tile([P, N], mybir.dt.float32,
                                           tag=f"ps{m}", name=f"ps{m}")
            for k in range(KT):
                for m in range(G0):
                    mm(m, k)
            for m in range(G0):
                evict(m)

            # Phase 2: remaining m-tiles m-major (inputs now resident),
            # copy-out pipelined with the next tile's matmuls.
            for m in range(G0, MT):
                ps_tiles[m % G0] = ps_pool.tile([P, N], mybir.dt.float32,
                                                tag=f"ps{m % G0}",
                                                name=f"ps{m}")
                for k in range(KT):
                    mm(m, k)
                evict(m)
    nc.finalize()
    return nc


def get_module():
    if "nc" not in _module_cache:
        _module_cache["nc"] = build_module()
    return _module_cache["nc"]


def _prepare_in_maps(x, kernel, scale):
    bf16 = ml_dtypes.bfloat16
    x2d = np.asarray(x, dtype=np.float32).reshape(ROWS, K)
    scale = np.float32(scale)
    w_signed = np.where(np.asarray(kernel, dtype=bool), scale, -scale)
    w_bf16 = np.ascontiguousarray(w_signed.astype(bf16))
    in_maps = []
    for c in range(N_CORES):
        shard = x2d[c * ROWS_PER_CORE:(c + 1) * ROWS_PER_CORE]
        xt_c = np.ascontiguousarray(shard.T.astype(bf16))
        in_maps.append({"xt": xt_c, "w": w_bf16})
    return in_maps


def kernel(x, kernel, scale):
    nc = get_module()
    in_maps = _prepare_in_maps(x, kernel, scale)
    res = run_bass_kernel_spmd(nc, in_maps, core_ids=list(range(N_CORES)))
    out = np.concatenate([r["out"] for r in res.results], axis=0)
    return out.reshape(B, S, N)
